# revision 1
# baseline (speedup 1.0000x reference)
"""NeuralMemory fast-weight recurrence on 8 Trainium2 NeuronCores — v2.

Sharding: 8-way tensor-parallel over memory dim M=2048 (MS=256/core).
One bf16 AllReduce of the layer-2 partial activation per chunk; `out` is
returned as per-core Q-space partial sums that the host scales (per-chunk
forget product c'_j), bias-corrects, and adds.

v2 changes vs v1:
- x supplied pre-transposed (host) in both R [T,D] and T [D,T] bf16 layouts:
  kills 32 PE-transposes + evacuations per chunk.
- Gate scalars (forget f_j, cumulative c_j, update scales) precomputed on the
  host from f32 x: kills the per-chunk gate matmuls/sigmoids/reductions.
- mm1 in zT form (stationary Q0 tiles, streaming xT): h comes out m-major so
  mm2 needs no h transpose; layer-1 bias applied as per-partition ACT bias.
- Critical-path-first emission: AR result -> dpred -> dh -> gW0 -> mm1 ->
  (gW1 early) -> mm2 -> next AR trigger; out-forward, gW1n, prefetch run in
  the AllReduce shadow.
- Layer-2 bias folded into the psum evacuation via a broadcast bias plane;
  out evacuated unscaled (host applies c' and bias).
"""
import numpy as np
import concourse.bacc as bacc
import concourse.mybir as mybir
import concourse.tile as tile
from concourse.bass_utils import run_bass_kernel_spmd

BF = mybir.dt.bfloat16
FP8 = mybir.dt.float8e4
F32 = mybir.dt.float32
AF = mybir.ActivationFunctionType
ALU = mybir.AluOpType

NCORES = 8
B, L, D, M = 2, 2048, 2048, 2048
C = 128                 # reference CHUNK
NCH = L // C            # 16 chunks
T = B * C               # 256 tokens per chunk
MS = M // NCORES        # 256 per-core memory slice
KD = D // 128           # 16 tiles over D
KT = T // 128           # 2 tiles over tokens
KM = MS // 128          # 2 tiles over m_s
NN = D // 512           # 4 N-chunks of 512 over D
LR_MEMORY = 0.01

# scal columns: [c, cn, f, negs, negs0, negs8, negs0p, 0]
SC_C, SC_CN, SC_F, SC_NEGS, SC_NEGS0, SC_NEGS8, SC_NEGS0P = range(7)


def build():
    nc = bacc.Bacc("TRN2", target_bir_lowering=False, num_devices=NCORES)
    xr_in = nc.dram_tensor("xr", [NCH, T, D], BF, kind="ExternalInput")
    xt_in = nc.dram_tensor("xt", [NCH, D, T], BF, kind="ExternalInput")
    w0t_in = nc.dram_tensor("w0t", [D, MS], F32, kind="ExternalInput")
    w1t_in = nc.dram_tensor("w1t", [MS, D], F32, kind="ExternalInput")
    w1n_in = nc.dram_tensor("w1n", [D, MS], F32, kind="ExternalInput")
    b0c_in = nc.dram_tensor("b0c", [128, KM], F32, kind="ExternalInput")
    b1d8_in = nc.dram_tensor("b1d8", [1, D], F32, kind="ExternalInput")
    scal_in = nc.dram_tensor("scal", [1, NCH * 8], F32, kind="ExternalInput")
    ident_in = nc.dram_tensor("ident", [128, 128], F32, kind="ExternalInput")
    outq = nc.dram_tensor("outq", [NCH, T, D], F32, kind="ExternalOutput")
    b1out = nc.dram_tensor("b1out", [NCH, D], F32, kind="ExternalOutput")

    with tile.TileContext(nc) as tc:
        with (
            tc.tile_pool(name="wp", bufs=1) as wp,            # persistent
            tc.tile_pool(name="xp", bufs=3) as xp,            # x streams
            tc.tile_pool(name="ap", bufs=2) as ap,            # loop-carried acts
            tc.tile_pool(name="tp", bufs=2) as tp,            # per-iter temps
            tc.tile_pool(name="sp", bufs=2) as spool,         # tiny tiles
            tc.tile_pool(name="gp", bufs=3) as gp,            # grad staging
            tc.tile_pool(name="psA", bufs=2, space="PSUM") as psA,   # [128,512]
            tc.tile_pool(name="psB", bufs=2, space="PSUM") as psB,   # [128,256]
            tc.tile_pool(name="psT", bufs=2, space="PSUM") as psT,   # [128,128]
            tc.tile_pool(name="psD", bufs=1, space="PSUM") as psD,   # small
            tc.tile_pool(name="dr", bufs=2, space="DRAM") as dr,
        ):
            # ---------------- persistent state ----------------
            q0t = wp.tile([128, KD * MS], BF, name="q0t")   # [d,m] d-tile i @ i*MS
            q1t = wp.tile([128, KM * D], BF, name="q1t")    # [m,d] m-tile k @ k*D
            q1n = wp.tile([128, KD * MS], BF, name="q1n")   # [d,m] d-tile i @ i*MS
            bp0 = wp.tile([128, KM], F32, name="bp0")       # P-space b0 columns
            bk1 = wp.tile([1, D], F32, name="bk1")          # Q-space b1/8 row
            ident = wp.tile([128, 128], BF, name="ident")
            ones_col = wp.tile([128, 1], BF, name="ones_col")
            ones_row = wp.tile([1, 128], BF, name="ones_row")
            junk = wp.tile([1, 1], BF, name="junk")
            bk1bf = wp.tile([1, D], BF, name="bk1bf")
            scal = wp.tile([1, NCH * 8], F32, name="scal")
            # per-chunk broadcast scalars [128,1] each: c, cn, f, negs, negs0
            cbc = wp.tile([128, NCH], F32, name="cbc")
            cnbc = wp.tile([128, NCH], F32, name="cnbc")
            fbc = wp.tile([128, NCH], F32, name="fbc")
            negsbc = wp.tile([128, NCH], F32, name="negsbc")
            negs0bc = wp.tile([128, NCH], F32, name="negs0bc")
            negs0pbc = wp.tile([128, NCH], F32, name="negs0pbc")

            nc.gpsimd.dma_start(
                q0t[:].rearrange("p (i m) -> p i m", m=MS),
                w0t_in[:].rearrange("(i p) m -> p i m", p=128))
            nc.gpsimd.dma_start(
                q1t[:].rearrange("p (k d) -> p k d", d=D),
                w1t_in[:].rearrange("(k p) d -> p k d", p=128))
            nc.gpsimd.dma_start(
                q1n[:].rearrange("p (i m) -> p i m", m=MS),
                w1n_in[:].rearrange("(i p) m -> p i m", p=128))
            nc.sync.dma_start(bp0[:], b0c_in[:])
            nc.sync.dma_start(bk1[:], b1d8_in[:])
            nc.gpsimd.dma_start(ident[:], ident_in[:])
            nc.vector.memset(ones_col[:], 1.0)
            nc.vector.memset(ones_row[:], 1.0)
            nc.scalar.copy(bk1bf[:], bk1[:])
            nc.sync.dma_start(scal[:], scal_in[:])
            for j in range(NCH):
                for dst, col in ((cbc, SC_C), (cnbc, SC_CN), (fbc, SC_F),
                                 (negsbc, SC_NEGS), (negs0bc, SC_NEGS0),
                                 (negs0pbc, SC_NEGS0P)):
                    nc.gpsimd.partition_broadcast(
                        dst[:, j:j + 1], scal[0:1, j * 8 + col:j * 8 + col + 1])

            # ---------------- helpers ----------------
            def load_x(j):
                # no casts needed (host supplies bf16) -> keep off the Pool
                # queue so prefetches never delay the collective trigger.
                xb = xp.tile([128, KT * D], BF, name=f"xb{j}", tag="xb")
                for k in range(KT):
                    nc.sync.dma_start(xb[:, k * D:(k + 1) * D],
                                      xr_in[j, k * 128:(k + 1) * 128, :])
                xT = xp.tile([128, KD * T], BF, name=f"xT{j}", tag="xT")
                nc.sync.dma_start(
                    xT[:].rearrange("p (i t) -> p i t", t=T),
                    xt_in[j].rearrange("(i p) t -> p i t", p=128))
                return xb, xT

            def mm1_zT(xT, j, cnc, want_dsilu, pfx):
                """zT-form layer 1: h[m,t'] under current q0t, scale cnc.
                If want_dsilu, returns (hT, hpT, emit_dsilu) — call emit_dsilu()
                later (hp is only needed next iteration; keep Dsilu off the
                critical path)."""
                hT = (ap if want_dsilu else tp).tile(
                    [128, KM * T], BF, name=f"h{pfx}_{j}", tag=f"h{pfx}")
                pts = []
                for a in range(KM):
                    pt = psB.tile([128, T], F32, name=f"z{pfx}_{j}_{a}", tag="psB")
                    for i in range(KD):
                        nc.tensor.matmul(
                            pt[:],
                            q0t[:, i * MS + a * 128:i * MS + (a + 1) * 128],
                            xT[:, i * T:(i + 1) * T],
                            start=(i == 0), stop=(i == KD - 1))
                    pts.append(pt)
                for a in range(KM):
                    nc.scalar.activation(hT[:, a * T:(a + 1) * T], pts[a][:],
                                         AF.Silu, bias=bp0[:, a:a + 1],
                                         scale=cnc)
                if not want_dsilu:
                    return hT, None
                hpT = ap.tile([128, KM * T], BF, name=f"hp{pfx}_{j}", tag="hp1")

                def emit_dsilu():
                    for a in range(KM):
                        nc.scalar.activation(hpT[:, a * T:(a + 1) * T], pts[a][:],
                                             AF.Derivative_silu,
                                             bias=bp0[:, a:a + 1], scale=cnc)
                return hT, hpT, emit_dsilu

            def mm2_R(hT, j, pfx, evac, with_bias=False):
                """R-form layer 2 partial: psums [t',512], evac(k, n, pt) inline."""
                for k in range(KT):
                    for n in range(NN):
                        pt = psA.tile([128, 512], F32,
                                      name=f"p{pfx}_{j}_{k}_{n}", tag="psA")
                        for a in range(KM):
                            nc.tensor.matmul(
                                pt[:],
                                hT[:, a * T + k * 128:a * T + (k + 1) * 128],
                                q1t[:, a * D + n * 512:a * D + (n + 1) * 512],
                                start=(a == 0),
                                stop=(a == KM - 1 and not with_bias))
                        if with_bias:
                            nc.tensor.matmul(
                                pt[:], ones_row[:],
                                bk1bf[0:1, n * 512:(n + 1) * 512],
                                start=False, stop=True)
                        evac(k, n, pt)

            def pe_t(dst, dst_col, src, src_col, nm, eng):
                pt = psT.tile([128, 128], BF, name=f"T{nm}", tag="psT")
                nc.tensor.transpose(pt[:], src[:, src_col:src_col + 128], ident[:])
                if eng is nc.scalar:
                    nc.scalar.copy(dst[:, dst_col:dst_col + 128], pt[:])
                else:
                    eng.tensor_copy(dst[:, dst_col:dst_col + 128], pt[:])
                return pt

            # ---------------- prologue: chunk 0 pred under P_0 ----------------
            xb_c, xT_c = load_x(0)
            c0 = cbc[:, 0:1]
            h1T_c, hp1T_c, dsilu0 = mm1_zT(xT_c, 0, c0, True, "1")
            predp = ap.tile([128, KT * D], FP8, name="predp0", tag="predp")

            def evac_pred(k, n, pt, dst, csc):
                # scale by the running forget product -> P-space partials stay
                # inside fp8e4 range (Q-space grows like 1/c and would overflow)
                sl = slice(k * D + n * 512, k * D + (n + 1) * 512)
                if n % 2 == 0:
                    nc.scalar.mul(dst[:, sl], pt[:], csc)
                else:
                    nc.vector.tensor_scalar_mul(dst[:, sl], pt[:], csc)

            mm2_R(h1T_c, 0, "p", lambda k, n, pt: evac_pred(k, n, pt, predp, c0),
                  with_bias=True)
            arin = dr.tile([T, D], FP8, name="arin0", tag="arin")
            for k in range(KT):
                nc.sync.dma_start(arin[k * 128:(k + 1) * 128, :],
                                  predp[:, k * D:(k + 1) * D])
            arout = dr.tile([T, D], FP8, name="arout0", tag="arout",
                            addr_space="Shared")
            nc.gpsimd.collective_compute(
                "AllReduce", ALU.add, replica_groups=[list(range(NCORES))],
                ins=[arin.opt()], outs=[arout.opt()])
            dsilu0()
            # h1R for gW1 of chunk 0 (in AR_0 shadow)
            h1R_c = ap.tile([128, KT * MS], BF, name="h1R0", tag="h1R")
            for a in range(KM):
                for k in range(KT):
                    pe_t(h1R_c, k * MS + a * 128, h1T_c, a * T + k * 128,
                         f"h0_{a}_{k}", nc.vector if (a + k) % 2 else nc.scalar)
            xb_n, xT_n = load_x(1)

            # ---------------- main loop ----------------
            for j in range(NCH):
                last = (j == NCH - 1)
                cj = cbc[:, j:j + 1]
                cnj = cnbc[:, j:j + 1]
                fj = fbc[:, j:j + 1]
                ngj = negsbc[:, j:j + 1]
                ng0j = negs0bc[:, j:j + 1]

                # -- AR_j result (P-space fp8) -> dpredR = pf - x --
                # gpsimd cast-DMA fp8->bf16 (Pool is idle at AR completion)
                pf = tp.tile([128, KT * D], BF, name=f"pf{j}", tag="pf")
                for k in range(KT):
                    for h in range(2):
                        nc.gpsimd.dma_start(
                            pf[:, k * D + h * 1024:k * D + (h + 1) * 1024],
                            arout[k * 128:(k + 1) * 128, h * 1024:(h + 1) * 1024])
                dpR = tp.tile([128, KT * D], BF, name=f"dpR{j}", tag="dpR")
                for q in range(4):
                    sl = slice(q * 1024, (q + 1) * 1024)
                    eng = nc.vector if q % 2 == 0 else nc.gpsimd
                    eng.tensor_tensor(dpR[:, sl], pf[:, sl], xb_c[:, sl],
                                      ALU.subtract)

                # -- dpredT via PE transpose (first: it gates dhT -> gW0 -> mm1) --
                dpT = tp.tile([128, KD * T], BF, name=f"dpT{j}", tag="dpT")
                engs = [nc.vector, nc.scalar]
                for k in range(KT):
                    for i in range(KD):
                        pe_t(dpT, i * T + k * 128, dpR, k * D + i * 128,
                             f"dp{j}_{k}_{i}", engs[(k * KD + i) % 2])

                # -- gW1 -> q1t update (overlaps the dpredT evac chase) --
                # GPSIMD can't read PSUM: ACT applies the scale (psum->bf16),
                # Pool accumulates into the weight (SBUF-SBUF).
                for a in range(KM):
                    for n in range(NN):
                        pt = psA.tile([128, 512], F32, name=f"g1_{j}_{a}_{n}",
                                      tag="psA")
                        for k in range(KT):
                            nc.tensor.matmul(
                                pt[:],
                                h1R_c[:, k * MS + a * 128:k * MS + (a + 1) * 128],
                                dpR[:, k * D + n * 512:k * D + (n + 1) * 512],
                                start=(k == 0), stop=(k == KT - 1))
                        sl = slice(a * D + n * 512, a * D + (n + 1) * 512)
                        if n % 2 == 0:
                            nc.vector.scalar_tensor_tensor(q1t[:, sl], pt[:], ngj,
                                                           q1t[:, sl], ALU.mult,
                                                           ALU.add)
                        else:
                            gq = gp.tile([128, 512], BF, name=f"gq1_{j}_{a}_{n}",
                                         tag="gq")
                            nc.scalar.mul(gq[:], pt[:], ngj)
                            nc.gpsimd.tensor_tensor(q1t[:, sl], q1t[:, sl], gq[:],
                                                    ALU.add)

                # -- dhT = W1 dpred^T [m,t] ; dzT = dhT * hp --
                dzT = tp.tile([128, KM * T], BF, name=f"dzT{j}", tag="dzT")
                for a in range(KM):
                    pt = psB.tile([128, T], F32, name=f"dh{j}_{a}", tag="psB")
                    for i in range(KD):
                        nc.tensor.matmul(
                            pt[:],
                            q1n[:, i * MS + a * 128:i * MS + (a + 1) * 128],
                            dpT[:, i * T:(i + 1) * T],
                            start=(i == 0), stop=(i == KD - 1))
                    nc.vector.tensor_tensor(dzT[:, a * T:(a + 1) * T], pt[:],
                                            hp1T_c[:, a * T:(a + 1) * T], ALU.mult)
                # dzR via PE transpose
                dzR = tp.tile([128, KT * MS], BF, name=f"dzR{j}", tag="dzR")
                for a in range(KM):
                    for k in range(KT):
                        pe_t(dzR, k * MS + a * 128, dzT, a * T + k * 128,
                             f"dz{j}_{a}_{k}", nc.vector if (a + k) % 2 else nc.scalar)

                # -- gW0 -> q0t update ; gb0 -> bp0 ; gb1 -> bk1 --
                for i in range(KD):
                    pt = psB.tile([128, MS], F32, name=f"g0_{j}_{i}", tag="psB")
                    for k in range(KT):
                        nc.tensor.matmul(
                            pt[:], xb_c[:, k * D + i * 128:k * D + (i + 1) * 128],
                            dzR[:, k * MS:(k + 1) * MS],
                            start=(k == 0), stop=(k == KT - 1))
                    sl = slice(i * MS, (i + 1) * MS)
                    if i % 2 == 0:
                        nc.vector.scalar_tensor_tensor(q0t[:, sl], pt[:], ng0j,
                                                       q0t[:, sl], ALU.mult, ALU.add)
                    else:
                        gq = gp.tile([128, MS], BF, name=f"gq0_{j}_{i}", tag="gq")
                        nc.scalar.mul(gq[:], pt[:], ng0j)
                        nc.gpsimd.tensor_tensor(q0t[:, sl], q0t[:, sl], gq[:],
                                                ALU.add)
                # gb0 columns [m,1] per m-tile: lhsT = dzR tiles, rhs = ones
                nc.vector.tensor_scalar_mul(bp0[:], bp0[:], fj)
                ng0pj = negs0pbc[:, j:j + 1]
                for a in range(KM):
                    pt = psD.tile([128, 1], F32, name=f"gb0_{j}_{a}", tag="psDc")
                    for k in range(KT):
                        nc.tensor.matmul(
                            pt[:], dzR[:, k * MS + a * 128:k * MS + (a + 1) * 128],
                            ones_col[:], start=(k == 0), stop=(k == KT - 1))
                    nc.vector.scalar_tensor_tensor(bp0[:, a:a + 1], pt[:], ng0pj,
                                                   bp0[:, a:a + 1], ALU.mult, ALU.add)
                # gb1 row; negs8 scalar via scal AP (row op, partition 0 only)
                for n in range(NN):
                    pt = psD.tile([1, 512], F32, name=f"gb1_{j}_{n}", tag="psDr")
                    for k in range(KT):
                        nc.tensor.matmul(
                            pt[:], ones_col[:],
                            dpR[:, k * D + n * 512:k * D + (n + 1) * 512],
                            start=(k == 0), stop=(k == KT - 1))
                    sl = slice(n * 512, (n + 1) * 512)
                    nc.vector.scalar_tensor_tensor(
                        bk1[0:1, sl], pt[:],
                        scal[0:1, j * 8 + SC_NEGS8:j * 8 + SC_NEGS8 + 1],
                        bk1[0:1, sl], ALU.mult, ALU.add)
                nc.sync.dma_start(b1out[j:j + 1, :], bk1[:])
                nc.scalar.copy(bk1bf[:], bk1[:])
                # prewarm the Silu ACT table during mm1's MM stream so the
                # real silu doesn't pay LoadActFuncSet on the critical path
                nc.scalar.activation(junk[:], scal[0:1, 0:1], AF.Silu)

                # -- forward chunk j+1 under P_{j+1}: pred -> AR (critical) --
                if not last:
                    h1T_n, hp1T_n, dsilu_n = mm1_zT(xT_n, j + 1, cnj, True, "1")
                    predp = ap.tile([128, KT * D], FP8, name=f"predp{j + 1}",
                                    tag="predp")
                    pp = predp
                    mm2_R(h1T_n, j + 1, "p",
                          lambda k, n, pt: evac_pred(k, n, pt, pp, cnj),
                          with_bias=True)
                    arin = dr.tile([T, D], FP8, name=f"arin{j + 1}", tag="arin")
                    for k in range(KT):
                        for h in range(2):
                            eng = nc.sync if h == 0 else nc.scalar
                            eng.dma_start(
                                arin[k * 128:(k + 1) * 128,
                                     h * 1024:(h + 1) * 1024],
                                predp[:, k * D + h * 1024:k * D + (h + 1) * 1024])
                    arout = dr.tile([T, D], FP8, name=f"arout{j + 1}", tag="arout",
                                    addr_space="Shared")
                    nc.gpsimd.collective_compute(
                        "AllReduce", ALU.add, replica_groups=[list(range(NCORES))],
                        ins=[arin.opt()], outs=[arout.opt()])
                    dsilu_n()

                # ---------- shadow of AR_{j+1} ----------
                # out_j forward under P_{j+1} (unscaled Q-space partial)
                h2T, _ = mm1_zT(xT_c, j, cnj, False, "2")
                outsb = tp.tile([128, KT * D], F32, name=f"o{j}", tag="outsb")

                def evac_out(k, n, pt, dst=outsb):
                    sl = slice(k * D + n * 512, k * D + (n + 1) * 512)
                    if n % 2 == 0:
                        nc.vector.tensor_copy(dst[:, sl], pt[:])
                    else:
                        nc.scalar.copy(dst[:, sl], pt[:])

                mm2_R(h2T, j, "o", evac_out)
                for k in range(KT):
                    nc.sync.dma_start(outq[j, k * 128:(k + 1) * 128, :],
                                      outsb[:, k * D:(k + 1) * D])

                # gW1n -> q1n update (needed as W1_{j+1} for next dhT)
                for i in range(KD):
                    pt = psB.tile([128, MS], F32, name=f"g1n_{j}_{i}", tag="psB")
                    for k in range(KT):
                        nc.tensor.matmul(
                            pt[:], dpR[:, k * D + i * 128:k * D + (i + 1) * 128],
                            h1R_c[:, k * MS:(k + 1) * MS],
                            start=(k == 0), stop=(k == KT - 1))
                    sl = slice(i * MS, (i + 1) * MS)
                    if i % 2 == 0:
                        nc.vector.scalar_tensor_tensor(q1n[:, sl], pt[:], ngj,
                                                       q1n[:, sl], ALU.mult, ALU.add)
                    else:
                        gq = gp.tile([128, MS], BF, name=f"gqn_{j}_{i}", tag="gq")
                        nc.scalar.mul(gq[:], pt[:], ngj)
                        nc.gpsimd.tensor_tensor(q1n[:, sl], q1n[:, sl], gq[:],
                                                ALU.add)

                if not last:
                    # h1R for next iteration's gW1
                    h1R_n = ap.tile([128, KT * MS], BF, name=f"h1R{j + 1}",
                                    tag="h1R")
                    for a in range(KM):
                        for k in range(KT):
                            pe_t(h1R_n, k * MS + a * 128, h1T_n, a * T + k * 128,
                                 f"h{j + 1}_{a}_{k}",
                                 nc.vector if (a + k) % 2 else nc.scalar)
                    if j + 2 < NCH:
                        xb_p, xT_p = load_x(j + 2)
                    xb_c, xT_c = xb_n, xT_n
                    if j + 2 < NCH:
                        xb_n, xT_n = xb_p, xT_p
                    h1T_c, hp1T_c, h1R_c = h1T_n, hp1T_n, h1R_n
    nc.compile()
    return nc


_NC_CACHE = None


def _get_nc():
    global _NC_CACHE
    if _NC_CACHE is None:
        _NC_CACHE = build()
    return _NC_CACHE


def _sigmoid(v):
    return 1.0 / (1.0 + np.exp(-v))


def host_prep(x, W0, b0, W1, b1, lr_w, lr_b, fg_w, fg_b):
    """Host-side: layouts, pre-transposed x, gate scalar schedule."""
    import ml_dtypes
    bf16 = ml_dtypes.bfloat16
    x = np.asarray(x, np.float32)
    # chunk layouts: xr [NCH, T, D], xt [NCH, D, T] (token index t = b*C + c)
    xch = np.transpose(x.reshape(B, NCH, C, D), (1, 0, 2, 3)).reshape(NCH, T, D)
    xr = np.ascontiguousarray(xch).astype(bf16)
    xt = np.ascontiguousarray(np.transpose(xch, (0, 2, 1))).astype(bf16)

    # gate scalars from f32 x (exact)
    lr_wv = np.asarray(lr_w, np.float32)[0]
    fg_wv = np.asarray(fg_w, np.float32)[0]
    lr_bv = float(np.asarray(lr_b, np.float32).reshape(-1)[0])
    fg_bv = float(np.asarray(fg_b, np.float32).reshape(-1)[0])
    scal = np.zeros((NCH, 8), np.float32)
    c = 1.0
    for j in range(NCH):
        ch = xch[j]                                   # (T, D)
        lsum = _sigmoid(ch @ lr_wv + lr_bv).sum()
        fparts = _sigmoid(ch.reshape(B, C, D).mean(axis=1) @ fg_wv + fg_bv)
        f = float(fparts.mean())
        cn = c * f
        negs = -LR_MEMORY * 2.0 * float(lsum) / (T * T * D) / cn
        scal[j] = [c, cn, f, negs, negs * c, negs / 8.0, negs * c * cn, 0.0]
        c = cn
    return xr, xt, scal


def make_in_maps(x, W0, b0, W1, b1, lr_w, lr_b, fg_w, fg_b):
    xr, xt, scal = host_prep(x, W0, b0, W1, b1, lr_w, lr_b, fg_w, fg_b)
    W0 = np.asarray(W0, np.float32)
    W1 = np.asarray(W1, np.float32)
    b0v = np.asarray(b0, np.float32)
    b1v = np.asarray(b1, np.float32)
    ident = np.eye(128, dtype=np.float32)
    in_maps = []
    for s in range(NCORES):
        sl = slice(s * MS, (s + 1) * MS)
        b0c = np.ascontiguousarray(b0v[sl].reshape(KM, 128).T)   # [128, KM]
        in_maps.append({
            "xr": xr,
            "xt": xt,
            "w0t": np.ascontiguousarray(W0[sl, :].T),
            "w1t": np.ascontiguousarray(W1[:, sl].T),
            "w1n": np.ascontiguousarray(W1[:, sl]),
            "b0c": b0c,
            "b1d8": np.ascontiguousarray((b1v / 8.0).reshape(1, D)),
            "scal": np.ascontiguousarray(scal.reshape(1, NCH * 8)),
            "ident": ident,
        })
    return in_maps


def run(inputs, **kw):
    nc = _get_nc()
    in_maps = make_in_maps(**inputs)
    res = run_bass_kernel_spmd(nc, in_maps, core_ids=list(range(NCORES)), **kw)
    scal = in_maps[0]["scal"].reshape(NCH, 8)
    outq = np.zeros((NCH, T, D), np.float32)
    for r in res.results:
        outq += r["outq"]
    b1rows = res.results[0]["b1out"]                  # [NCH, D] = qb1/8 rows
    cn = scal[:, SC_CN].reshape(NCH, 1, 1)
    outq = cn * outq + (cn * 8.0) * b1rows.reshape(NCH, 1, D)
    out = np.ascontiguousarray(
        np.transpose(outq.reshape(NCH, B, C, D), (1, 0, 2, 3))).reshape(B, L, D)
    return out, res


def kernel(**inputs) -> np.ndarray:
    out, _ = run(inputs)
    return out



# revision 8
# speedup vs baseline: 1.1868x; 1.1868x over previous
"""NeuralMemory fast-weight recurrence on 8 Trainium2 NeuronCores — v3.

Sharding: 8-way tensor-parallel over memory dim M=2048 (MS=256/core).
One fp8 AllReduce per chunk whose payload is c*(pred partial + qb1/8) - x/8,
so the AR output IS dpred directly (bias and target folded into the
contributions; sum over 8 cores telescopes to pred + b1 - x).

v3 changes vs v2:
- dpred-in-AR: kills the post-AR subtract; arout cast-loads straight into dpR.
- All PE-transpose evacuations batched: 8 transposes share one [128,1024]
  bf16 psum tile, evacuated by a single DVE/ACT op (4 ops for dpT instead
  of 32).
- Gradient psums packed in [128,512] pairs -> half the update-evac ops.
- outq written in bf16 (host sums partials in f32).
- PE order in the critical section: dpT -> dh -> dzR -> gW0 -> gW1 -> mm1
  -> mm2 (q0t/q1t updates overlap the following matmuls).
- gpsimd issues nothing after the collective trigger except the next
  chunk's post-AR cast, so an engine-blocking collective costs nothing.
"""
import numpy as np
import concourse.bacc as bacc
import concourse.mybir as mybir
import concourse.tile as tile
from concourse.bass_utils import run_bass_kernel_spmd

BF = mybir.dt.bfloat16
FP8 = mybir.dt.float8e4
F32 = mybir.dt.float32
AF = mybir.ActivationFunctionType
ALU = mybir.AluOpType

NCORES = 8
B, L, D, M = 2, 2048, 2048, 2048
C = 128                 # reference CHUNK
NCH = L // C            # 16 chunks
T = B * C               # 256 tokens per chunk
MS = M // NCORES        # 256 per-core memory slice
KD = D // 128           # 16 tiles over D
KT = T // 128           # 2 tiles over tokens
KM = MS // 128          # 2 tiles over m_s
NN = D // 512           # 4 N-chunks of 512 over D
LR_MEMORY = 0.01

# scal columns: [c, cn, f, negs, negs0x8, negs8, negs0p, 0]
SC_C, SC_CN, SC_F, SC_NEGS, SC_NEGS0, SC_NEGS8, SC_NEGS0P = range(7)


def build(ar=True):
    nc = bacc.Bacc("TRN2", target_bir_lowering=False, num_devices=NCORES)
    xr8_in = nc.dram_tensor("xr8", [NCH, T, D], BF, kind="ExternalInput")
    xt_in = nc.dram_tensor("xt", [NCH, D, T], BF, kind="ExternalInput")
    w0t_in = nc.dram_tensor("w0t", [D, MS], F32, kind="ExternalInput")
    w1t_in = nc.dram_tensor("w1t", [MS, D], F32, kind="ExternalInput")
    w1n_in = nc.dram_tensor("w1n", [D, MS], F32, kind="ExternalInput")
    b0c_in = nc.dram_tensor("b0c", [128, KM], F32, kind="ExternalInput")
    b1d8_in = nc.dram_tensor("b1d8", [1, D], F32, kind="ExternalInput")
    scal_in = nc.dram_tensor("scal", [1, NCH * 8], F32, kind="ExternalInput")
    ident_in = nc.dram_tensor("ident", [128, 128], F32, kind="ExternalInput")
    outq = nc.dram_tensor("outq", [NCH, T, D], BF, kind="ExternalOutput")
    b1out = nc.dram_tensor("b1out", [NCH, D], F32, kind="ExternalOutput")

    with tile.TileContext(nc) as tc:
        with (
            tc.tile_pool(name="wp", bufs=1) as wp,            # persistent
            tc.tile_pool(name="xp", bufs=3) as xp,            # x streams
            tc.tile_pool(name="ap", bufs=2) as ap,            # loop-carried acts
            tc.tile_pool(name="tp", bufs=2) as tp,            # per-iter temps
            tc.tile_pool(name="psA", bufs=2, space="PSUM") as psA,   # [128,512]
            tc.tile_pool(name="psB", bufs=2, space="PSUM") as psB,   # [128,256]
            tc.tile_pool(name="psT", bufs=2, space="PSUM") as psT,   # [128,1024] BF
            tc.tile_pool(name="psD", bufs=1, space="PSUM") as psD,   # small
            tc.tile_pool(name="dr", bufs=2, space="DRAM") as dr,
        ):
            # ---------------- persistent state ----------------
            q0t = wp.tile([128, KD * MS], BF, name="q0t")   # [d,m] d-tile i @ i*MS
            q1t = wp.tile([128, KM * D], BF, name="q1t")    # [m,d] m-tile k @ k*D
            q1n = wp.tile([128, KD * MS], BF, name="q1n")   # [d,m] d-tile i @ i*MS
            bp0 = wp.tile([128, KM], F32, name="bp0")       # P-space b0 columns
            bk1 = wp.tile([1, D], F32, name="bk1")          # Q-space b1/8 row
            ident = wp.tile([128, 128], BF, name="ident")
            ones_col = wp.tile([128, 1], BF, name="ones_col")
            ones_row = wp.tile([1, 128], BF, name="ones_row")
            junk = wp.tile([1, 1], BF, name="junk")
            bk1bf = wp.tile([1, D], BF, name="bk1bf")
            scal = wp.tile([1, NCH * 8], F32, name="scal")
            cbc = wp.tile([128, NCH], F32, name="cbc")
            cnbc = wp.tile([128, NCH], F32, name="cnbc")
            fbc = wp.tile([128, NCH], F32, name="fbc")
            negsbc = wp.tile([128, NCH], F32, name="negsbc")
            negs0bc = wp.tile([128, NCH], F32, name="negs0bc")
            negs0pbc = wp.tile([128, NCH], F32, name="negs0pbc")

            nc.gpsimd.dma_start(
                q0t[:].rearrange("p (i m) -> p i m", m=MS),
                w0t_in[:].rearrange("(i p) m -> p i m", p=128))
            nc.gpsimd.dma_start(
                q1t[:].rearrange("p (k d) -> p k d", d=D),
                w1t_in[:].rearrange("(k p) d -> p k d", p=128))
            nc.gpsimd.dma_start(
                q1n[:].rearrange("p (i m) -> p i m", m=MS),
                w1n_in[:].rearrange("(i p) m -> p i m", p=128))
            nc.sync.dma_start(bp0[:], b0c_in[:])
            nc.sync.dma_start(bk1[:], b1d8_in[:])
            nc.gpsimd.dma_start(ident[:], ident_in[:])
            nc.vector.memset(ones_col[:], 1.0)
            nc.vector.memset(ones_row[:], 1.0)
            nc.scalar.copy(bk1bf[:], bk1[:])
            nc.sync.dma_start(scal[:], scal_in[:])
            for j in range(NCH):
                for dst, col in ((cbc, SC_C), (cnbc, SC_CN), (fbc, SC_F),
                                 (negsbc, SC_NEGS), (negs0bc, SC_NEGS0),
                                 (negs0pbc, SC_NEGS0P)):
                    nc.gpsimd.partition_broadcast(
                        dst[:, j:j + 1], scal[0:1, j * 8 + col:j * 8 + col + 1])

            # ---------------- helpers ----------------
            def load_x(j):
                xb = xp.tile([128, KT * D], BF, name=f"xb{j}", tag="xb")
                for k in range(KT):
                    nc.sync.dma_start(xb[:, k * D:(k + 1) * D],
                                      xr8_in[j, k * 128:(k + 1) * 128, :])
                xT = xp.tile([128, KD * T], BF, name=f"xT{j}", tag="xT")
                nc.sync.dma_start(
                    xT[:].rearrange("p (i t) -> p i t", t=T),
                    xt_in[j].rearrange("(i p) t -> p i t", p=128))
                return xb, xT

            def mm1_zT(xT, j, cnc, want_dsilu, pfx):
                """zT-form layer 1: h[m,t'] under current q0t, scale cnc."""
                hT = (ap if want_dsilu else tp).tile(
                    [128, KM * T], BF, name=f"h{pfx}_{j}", tag=f"h{pfx}")
                pts = []
                for a in range(KM):
                    pt = psB.tile([128, T], F32, name=f"z{pfx}_{j}_{a}", tag="psB")
                    for i in range(KD):
                        nc.tensor.matmul(
                            pt[:],
                            q0t[:, i * MS + a * 128:i * MS + (a + 1) * 128],
                            xT[:, i * T:(i + 1) * T],
                            start=(i == 0), stop=(i == KD - 1))
                    pts.append(pt)
                for a in range(KM):
                    nc.scalar.activation(hT[:, a * T:(a + 1) * T], pts[a][:],
                                         AF.Silu, bias=bp0[:, a:a + 1],
                                         scale=cnc)
                if not want_dsilu:
                    return hT, None
                hpT = ap.tile([128, KM * T], BF, name=f"hp{pfx}_{j}", tag="hp1")

                def emit_dsilu():
                    for a in range(KM):
                        nc.scalar.activation(hpT[:, a * T:(a + 1) * T], pts[a][:],
                                             AF.Derivative_silu,
                                             bias=bp0[:, a:a + 1], scale=cnc)
                return hT, hpT, emit_dsilu

            def mm2_R(hT, j, pfx, evac, with_bias=False):
                """R-form layer 2 partial: psums [t',512], evac(k, n, pt) inline."""
                for k in range(KT):
                    for n in range(NN):
                        pt = psA.tile([128, 512], F32,
                                      name=f"p{pfx}_{j}_{k}_{n}", tag="psA")
                        for a in range(KM):
                            nc.tensor.matmul(
                                pt[:],
                                hT[:, a * T + k * 128:a * T + (k + 1) * 128],
                                q1t[:, a * D + n * 512:a * D + (n + 1) * 512],
                                start=(a == 0),
                                stop=(a == KM - 1 and not with_bias))
                        if with_bias:
                            nc.tensor.matmul(
                                pt[:], ones_row[:],
                                bk1bf[0:1, n * 512:(n + 1) * 512],
                                start=False, stop=True)
                        evac(k, n, pt)

            def evac_pred(k, n, pt, dst, csc, xb):
                # dst = c*psum - x/8  (q-space pred partial + qb1/8, scaled to
                # P-space, minus the target share) -> AR sums to dpred.
                sl = slice(k * D + n * 512, k * D + (n + 1) * 512)
                nc.vector.scalar_tensor_tensor(dst[:, sl], pt[:], csc,
                                               xb[:, sl], ALU.mult,
                                               ALU.subtract)

            def send_ar(predp, j):
                arin = dr.tile([T, D], FP8, name=f"arin{j}", tag="arin")
                for k in range(KT):
                    for h in range(2):
                        eng = nc.sync if h == 0 else nc.scalar
                        eng.dma_start(
                            arin[k * 128:(k + 1) * 128,
                                 h * 1024:(h + 1) * 1024],
                            predp[:, k * D + h * 1024:k * D + (h + 1) * 1024])
                if ar:
                    arout = dr.tile([T, D], FP8, name=f"arout{j}", tag="arout",
                                    addr_space="Shared")
                    nc.gpsimd.collective_compute(
                        "AllReduce", ALU.add, replica_groups=[list(range(NCORES))],
                        ins=[arin.opt()], outs=[arout.opt()])
                else:
                    arout = arin
                return arout

            # ---------------- prologue: chunk 0 pred under P_0 ----------------
            xb_c, xT_c = load_x(0)
            c0 = cbc[:, 0:1]
            h1T_c, hp1T_c, dsilu0 = mm1_zT(xT_c, 0, c0, True, "1")
            predp = ap.tile([128, KT * D], FP8, name="predp0", tag="predp")
            mm2_R(h1T_c, 0, "p",
                  lambda k, n, pt: evac_pred(k, n, pt, predp, c0, xb_c),
                  with_bias=True)
            arout = send_ar(predp, 0)
            dsilu0()
            # h1R for gW1 of chunk 0 (in AR_0 shadow): batched transpose evac
            h1R_c = ap.tile([128, KT * MS], BF, name="h1R0", tag="h1R")
            pt = psT.tile([128, 1024], BF, name="Th0", tag="psT")
            for k in range(KT):
                for a in range(KM):
                    nc.tensor.transpose(pt[:, (k * KM + a) * 128:
                                            (k * KM + a + 1) * 128],
                                        h1T_c[:, a * T + k * 128:
                                              a * T + (k + 1) * 128], ident[:])
            nc.scalar.copy(h1R_c[:], pt[:, 0:KT * MS])
            xb_n, xT_n = load_x(1)

            # ---------------- main loop ----------------
            for j in range(NCH):
                last = (j == NCH - 1)
                cnj = cnbc[:, j:j + 1]
                fj = fbc[:, j:j + 1]
                ngj = negsbc[:, j:j + 1]
                ng0j = negs0bc[:, j:j + 1]

                # -- AR_j result IS dpred (P-space). cast fp8 -> bf16 --
                dpR = tp.tile([128, KT * D], BF, name=f"dpR{j}", tag="dpR")
                for h in range(2):          # d-low halves first for both k
                    for k in range(KT):
                        nc.gpsimd.dma_start(
                            dpR[:, k * D + h * 1024:k * D + (h + 1) * 1024],
                            arout[k * 128:(k + 1) * 128, h * 1024:(h + 1) * 1024])

                # -- dpT via batched PE transposes (4 groups of 8) --
                dpT = tp.tile([128, KD * T], BF, name=f"dpT{j}", tag="dpT")
                for g in range(4):
                    pt = psT.tile([128, 1024], BF, name=f"TdP{j}_{g}", tag="psT")
                    for x in range(8):
                        i, k = (g * 8 + x) // KT, (g * 8 + x) % KT
                        nc.tensor.transpose(
                            pt[:, x * 128:(x + 1) * 128],
                            dpR[:, k * D + i * 128:k * D + (i + 1) * 128],
                            ident[:])
                    # psum cols (i_local, k, c) flatten to the same linear
                    # order as dpT's [i, t=k*128+c] free range for this group
                    eng = nc.vector if g % 2 else nc.scalar
                    if eng is nc.scalar:
                        nc.scalar.copy(dpT[:, g * 1024:(g + 1) * 1024], pt[:])
                    else:
                        eng.tensor_copy(dpT[:, g * 1024:(g + 1) * 1024], pt[:])

                # -- dhT = W1 dpred^T [m,t] ; dzT = dhT * hp --
                dzT = tp.tile([128, KM * T], BF, name=f"dzT{j}", tag="dzT")
                for a in range(KM):
                    pt = psB.tile([128, T], F32, name=f"dh{j}_{a}", tag="psB")
                    for i in range(KD):
                        nc.tensor.matmul(
                            pt[:],
                            q1n[:, i * MS + a * 128:i * MS + (a + 1) * 128],
                            dpT[:, i * T:(i + 1) * T],
                            start=(i == 0), stop=(i == KD - 1))
                    nc.vector.tensor_tensor(dzT[:, a * T:(a + 1) * T], pt[:],
                                            hp1T_c[:, a * T:(a + 1) * T], ALU.mult)
                # dzR via batched transpose (one group of 4)
                dzR = tp.tile([128, KT * MS], BF, name=f"dzR{j}", tag="dzR")
                pt = psT.tile([128, 1024], BF, name=f"Tdz{j}", tag="psT")
                for a in range(KM):
                    for k in range(KT):
                        nc.tensor.transpose(pt[:, (k * KM + a) * 128:
                                                (k * KM + a + 1) * 128],
                                            dzT[:, a * T + k * 128:
                                                a * T + (k + 1) * 128], ident[:])
                nc.vector.tensor_copy(dzR[:], pt[:, 0:KT * MS])

                # -- gW0 -> q0t update (paired psums) ; gb0 ; gb1 --
                for i2 in range(KD // 2):
                    pt = psA.tile([128, 512], F32, name=f"g0_{j}_{i2}", tag="psA")
                    for ii in range(2):
                        i = i2 * 2 + ii
                        for k in range(KT):
                            nc.tensor.matmul(
                                pt[:, ii * MS:(ii + 1) * MS],
                                xb_c[:, k * D + i * 128:k * D + (i + 1) * 128],
                                dzR[:, k * MS:(k + 1) * MS],
                                start=(k == 0), stop=(k == KT - 1))
                    sl = slice(i2 * 512, (i2 + 1) * 512)
                    nc.vector.scalar_tensor_tensor(q0t[:, sl], pt[:], ng0j,
                                                   q0t[:, sl], ALU.mult, ALU.add)
                nc.vector.tensor_scalar_mul(bp0[:], bp0[:], fj)
                ng0pj = negs0pbc[:, j:j + 1]
                for a in range(KM):
                    pt = psD.tile([128, 1], F32, name=f"gb0_{j}_{a}", tag="psDc")
                    for k in range(KT):
                        nc.tensor.matmul(
                            pt[:], dzR[:, k * MS + a * 128:k * MS + (a + 1) * 128],
                            ones_col[:], start=(k == 0), stop=(k == KT - 1))
                    nc.vector.scalar_tensor_tensor(bp0[:, a:a + 1], pt[:], ng0pj,
                                                   bp0[:, a:a + 1], ALU.mult, ALU.add)
                for n in range(NN):
                    pt = psD.tile([1, 512], F32, name=f"gb1_{j}_{n}", tag="psDr")
                    for k in range(KT):
                        nc.tensor.matmul(
                            pt[:], ones_col[:],
                            dpR[:, k * D + n * 512:k * D + (n + 1) * 512],
                            start=(k == 0), stop=(k == KT - 1))
                    sl = slice(n * 512, (n + 1) * 512)
                    nc.vector.scalar_tensor_tensor(
                        bk1[0:1, sl], pt[:],
                        scal[0:1, j * 8 + SC_NEGS8:j * 8 + SC_NEGS8 + 1],
                        bk1[0:1, sl], ALU.mult, ALU.add)
                nc.sync.dma_start(b1out[j:j + 1, :], bk1[:])
                nc.scalar.copy(bk1bf[:], bk1[:])

                # -- gW1 -> q1t update (runs on PE while q0t update evacs) --
                for a in range(KM):
                    for n in range(NN):
                        pt = psA.tile([128, 512], F32, name=f"g1_{j}_{a}_{n}",
                                      tag="psA")
                        for k in range(KT):
                            nc.tensor.matmul(
                                pt[:],
                                h1R_c[:, k * MS + a * 128:k * MS + (a + 1) * 128],
                                dpR[:, k * D + n * 512:k * D + (n + 1) * 512],
                                start=(k == 0), stop=(k == KT - 1))
                        sl = slice(a * D + n * 512, a * D + (n + 1) * 512)
                        if n % 2 == 0:
                            nc.vector.scalar_tensor_tensor(q1t[:, sl], pt[:], ngj,
                                                           q1t[:, sl], ALU.mult,
                                                           ALU.add)
                        else:
                            gq = tp.tile([128, 512], BF, name=f"gq1_{j}_{a}_{n}",
                                         tag="gq")
                            nc.scalar.mul(gq[:], pt[:], ngj)
                            nc.gpsimd.tensor_tensor(q1t[:, sl], q1t[:, sl], gq[:],
                                                    ALU.add)
                # prewarm the Silu ACT table before the critical silu
                nc.scalar.activation(junk[:], scal[0:1, 0:1], AF.Silu)

                # -- forward chunk j+1 under P_{j+1}: pred -> AR (critical) --
                if not last:
                    h1T_n, hp1T_n, dsilu_n = mm1_zT(xT_n, j + 1, cnj, True, "1")
                    predp = ap.tile([128, KT * D], FP8, name=f"predp{j + 1}",
                                    tag="predp")
                    pp = predp
                    xbn = xb_n
                    mm2_R(h1T_n, j + 1, "p",
                          lambda k, n, pt: evac_pred(k, n, pt, pp, cnj, xbn),
                          with_bias=True)
                    arout = send_ar(predp, j + 1)
                    dsilu_n()

                # ---------- shadow of AR_{j+1} ----------
                # out_j forward under P_{j+1} (unscaled Q-space partial)
                h2T, _ = mm1_zT(xT_c, j, cnj, False, "2")
                outsb = tp.tile([128, KT * D], BF, name=f"o{j}", tag="outsb")

                def evac_out(k, n, pt, dst=outsb):
                    sl = slice(k * D + n * 512, k * D + (n + 1) * 512)
                    if n % 2 == 0:
                        nc.vector.tensor_copy(dst[:, sl], pt[:])
                    else:
                        nc.scalar.copy(dst[:, sl], pt[:])

                mm2_R(h2T, j, "o", evac_out)
                for k in range(KT):
                    nc.sync.dma_start(outq[j, k * 128:(k + 1) * 128, :],
                                      outsb[:, k * D:(k + 1) * D])

                # gW1n -> q1n update (paired psums, vector-only evac: gpsimd
                # may be blocked on the AR wait)
                for i2 in range(KD // 2):
                    pt = psA.tile([128, 512], F32, name=f"g1n_{j}_{i2}", tag="psA")
                    for ii in range(2):
                        i = i2 * 2 + ii
                        for k in range(KT):
                            nc.tensor.matmul(
                                pt[:, ii * MS:(ii + 1) * MS],
                                dpR[:, k * D + i * 128:k * D + (i + 1) * 128],
                                h1R_c[:, k * MS:(k + 1) * MS],
                                start=(k == 0), stop=(k == KT - 1))
                    sl = slice(i2 * 512, (i2 + 1) * 512)
                    nc.vector.scalar_tensor_tensor(q1n[:, sl], pt[:], ngj,
                                                   q1n[:, sl], ALU.mult, ALU.add)

                if not last:
                    # h1R for next iteration's gW1 (batched transpose)
                    h1R_n = ap.tile([128, KT * MS], BF, name=f"h1R{j + 1}",
                                    tag="h1R")
                    pt = psT.tile([128, 1024], BF, name=f"Th{j + 1}", tag="psT")
                    for k in range(KT):
                        for a in range(KM):
                            nc.tensor.transpose(pt[:, (k * KM + a) * 128:
                                                    (k * KM + a + 1) * 128],
                                                h1T_n[:, a * T + k * 128:
                                                      a * T + (k + 1) * 128],
                                                ident[:])
                    nc.scalar.copy(h1R_n[:], pt[:, 0:KT * MS])
                    if j + 2 < NCH:
                        xb_p, xT_p = load_x(j + 2)
                    xb_c, xT_c = xb_n, xT_n
                    if j + 2 < NCH:
                        xb_n, xT_n = xb_p, xT_p
                    h1T_c, hp1T_c, h1R_c = h1T_n, hp1T_n, h1R_n
    nc.compile()
    return nc


_NC_CACHE = None


def _get_nc():
    global _NC_CACHE
    if _NC_CACHE is None:
        _NC_CACHE = build()
    return _NC_CACHE


def _sigmoid(v):
    return 1.0 / (1.0 + np.exp(-v))


def host_prep(x, W0, b0, W1, b1, lr_w, lr_b, fg_w, fg_b):
    """Host-side: layouts, pre-transposed x, gate scalar schedule."""
    import ml_dtypes
    bf16 = ml_dtypes.bfloat16
    x = np.asarray(x, np.float32)
    # chunk layouts: xr8 [NCH, T, D] = x/8, xt [NCH, D, T] (token t = b*C + c)
    xch = np.transpose(x.reshape(B, NCH, C, D), (1, 0, 2, 3)).reshape(NCH, T, D)
    xr8 = np.ascontiguousarray(xch / 8.0).astype(bf16)
    xt = np.ascontiguousarray(np.transpose(xch, (0, 2, 1))).astype(bf16)

    lr_wv = np.asarray(lr_w, np.float32)[0]
    fg_wv = np.asarray(fg_w, np.float32)[0]
    lr_bv = float(np.asarray(lr_b, np.float32).reshape(-1)[0])
    fg_bv = float(np.asarray(fg_b, np.float32).reshape(-1)[0])
    scal = np.zeros((NCH, 8), np.float32)
    c = 1.0
    for j in range(NCH):
        ch = xch[j]                                   # (T, D)
        lsum = _sigmoid(ch @ lr_wv + lr_bv).sum()
        fparts = _sigmoid(ch.reshape(B, C, D).mean(axis=1) @ fg_wv + fg_bv)
        f = float(fparts.mean())
        cn = c * f
        negs = -LR_MEMORY * 2.0 * float(lsum) / (T * T * D) / cn
        # negs0 gets x8: gW0's lhsT streams x/8 instead of x
        scal[j] = [c, cn, f, negs, negs * c * 8.0, negs / 8.0, negs * c * cn, 0.0]
        c = cn
    return xr8, xt, scal


def make_in_maps(x, W0, b0, W1, b1, lr_w, lr_b, fg_w, fg_b):
    xr8, xt, scal = host_prep(x, W0, b0, W1, b1, lr_w, lr_b, fg_w, fg_b)
    W0 = np.asarray(W0, np.float32)
    W1 = np.asarray(W1, np.float32)
    b0v = np.asarray(b0, np.float32)
    b1v = np.asarray(b1, np.float32)
    ident = np.eye(128, dtype=np.float32)
    in_maps = []
    for s in range(NCORES):
        sl = slice(s * MS, (s + 1) * MS)
        b0c = np.ascontiguousarray(b0v[sl].reshape(KM, 128).T)   # [128, KM]
        in_maps.append({
            "xr8": xr8,
            "xt": xt,
            "w0t": np.ascontiguousarray(W0[sl, :].T),
            "w1t": np.ascontiguousarray(W1[:, sl].T),
            "w1n": np.ascontiguousarray(W1[:, sl]),
            "b0c": b0c,
            "b1d8": np.ascontiguousarray((b1v / 8.0).reshape(1, D)),
            "scal": np.ascontiguousarray(scal.reshape(1, NCH * 8)),
            "ident": ident,
        })
    return in_maps


def run(inputs, **kw):
    nc = _get_nc()
    in_maps = make_in_maps(**inputs)
    res = run_bass_kernel_spmd(nc, in_maps, core_ids=list(range(NCORES)), **kw)
    scal = in_maps[0]["scal"].reshape(NCH, 8)
    outq = np.zeros((NCH, T, D), np.float32)
    for r in res.results:
        outq += np.asarray(r["outq"], dtype=np.float32)
    b1rows = res.results[0]["b1out"]                  # [NCH, D] = qb1/8 rows
    cn = scal[:, SC_CN].reshape(NCH, 1, 1)
    outq = cn * outq + (cn * 8.0) * b1rows.reshape(NCH, 1, D)
    out = np.ascontiguousarray(
        np.transpose(outq.reshape(NCH, B, C, D), (1, 0, 2, 3))).reshape(B, L, D)
    return out, res


def kernel(**inputs) -> np.ndarray:
    out, _ = run(inputs)
    return out
